# revision 20
# baseline (speedup 1.0000x reference)
"""Multi-head self-attention Trainium2 kernel, sharded over 8 NeuronCores.

Sharding: core = (batch, head_group): 2 batches x 4 head-groups (4 heads each).
Each core computes qkv for its batch restricted to its heads, full-sequence
attention for those heads, and a row-parallel slice of the output projection,
producing a partial [T, C] output (fp16). Host: out[b] = sum of the 4
head-group partials + b_eff where b_eff folds b_proj and the V bias.

v2 design notes (all relative to the fp32/on-chip-transpose baseline):
  - x is transposed, packed and cast to fp16 on the host; no on-chip
    transposes or x^T copies are needed.
  - K bias is dropped entirely (softmax is invariant to per-query constants,
    and q.bk is per-query); V bias is folded into b_proj on the host
    (sum_s w_s = 1); only the Q bias is applied on-chip.
  - AV is computed transposed: out[q, d] = sum_s P[s,q] V[s,d] with
    ap_size=65 per chunk matmul, which halves the PE cost of AV and makes
    the softmax divide a single per-partition tensor_scalar divide.
  - The softmax denominator comes from a ones-column appended per head in
    the V tile (memset once).
  - Everything on the PE runs fp16 (1.0 cycles/row); fp8 was measured to
    break the 2e-2 gate (diffuse attention preserves per-key noise).
"""

import math
import sys

import numpy as np

sys.path.insert(0, "/opt/trn_rl_repo")

import concourse.bacc as bacc
import concourse.bass as bass
import concourse.tile as tile
from concourse import mybir
from concourse.bass_utils import run_bass_kernel_spmd

B, T, C = 2, 2048, 1024
NH, DH = 16, 64
HG = 4                  # heads per core
DL = HG * DH            # 256 local head dims
N_CORES = 8

F32 = mybir.dt.float32
F16 = mybir.dt.float16

SCALE = 1.0 / math.sqrt(DH)
Exp = mybir.ActivationFunctionType.Exp


def build_bass():
    nc = bacc.Bacc("TRN2", target_bir_lowering=False, debug=False)

    # host-packed params: [p, ci*w + j] = w[ci*128 + p, j]
    x_in = nc.declare_dram_parameter("x_pack", [128, 8 * T], F16, isOutput=False)
    wk_in = nc.declare_dram_parameter("wk_pack", [128, 8 * DL], F16, isOutput=False)
    wq_in = nc.declare_dram_parameter("wq_pack", [128, 8 * DL], F16, isOutput=False)
    wv_in = nc.declare_dram_parameter("wv_pack", [128, 8 * DL], F16, isOutput=False)
    wp_in = nc.declare_dram_parameter("wp_pack", [128, 2 * C], F16, isOutput=False)
    bq_in = nc.declare_dram_parameter("b_q", [128, 2], F32, isOutput=False)
    id_in = nc.declare_dram_parameter("iden16", [128, 128], F16, isOutput=False)
    out = nc.declare_dram_parameter("out_partial", [T, C], F16, isOutput=True)

    with tile.TileContext(nc) as tc:
        with (
            tc.tile_pool(name="singles", bufs=1) as singles,
            tc.tile_pool(name="pt", bufs=44) as ptp,
            tc.tile_pool(name="osb", bufs=6) as osbp,
            tc.tile_pool(name="oout", bufs=3) as ooutp,
            tc.tile_pool(name="sc", bufs=2, space="PSUM") as pssc,     # 2x2 banks
            tc.tile_pool(name="avp", bufs=1, space="PSUM") as psav,    # 1 bank
            tc.tile_pool(name="mm", bufs=3, space="PSUM") as psmm,     # 3x1 bank
        ):
            # ---- persistent sbuf tiles ---------------------------------
            warm = singles.tile([128, 512], F16, name="warm")
            nc.vector.memset(warm[:], 0.0)
            # pre-load the Exp activation table while DMAs are in flight
            warm_exp = singles.tile([128, 1], F16, name="warm_exp")
            nc.scalar.activation(warm_exp[:], warm[:, 0:1], Exp, scale=SCALE)

            xt = singles.tile([128, 8 * T], F16, name="xt")
            xt3 = xt[:].rearrange("p (ci t) -> p ci t", ci=8)
            xsrc = x_in[:].rearrange("p (ci t) -> p ci t", ci=8)
            NSL = 8
            TSL = T // NSL

            def x_slice(s):
                nc.sync.dma_start(
                    out=xt3[:, :, s * TSL:(s + 1) * TSL],
                    in_=xsrc[:, :, s * TSL:(s + 1) * TSL],
                )

            # DMA order tuned so the first score tile unblocks earliest:
            # Q projection (wq + x s0,s1) is the long pole for score p0.
            wq = singles.tile([128, 8 * DL], F16, name="wq")
            nc.sync.dma_start(out=wq[:], in_=wq_in[:])
            bq = singles.tile([128, 2], F32, name="bq")
            nc.sync.dma_start(out=bq[:], in_=bq_in[:])
            x_slice(0)
            wk = singles.tile([128, 8 * DL], F16, name="wk")
            nc.sync.dma_start(out=wk[:], in_=wk_in[:])
            x_slice(1)
            x_slice(2)
            x_slice(3)
            wv = singles.tile([128, 8 * DL], F16, name="wv")
            nc.sync.dma_start(out=wv[:], in_=wv_in[:])
            x_slice(4)
            x_slice(5)
            x_slice(6)
            x_slice(7)
            wp = singles.tile([128, 2 * C], F16, name="wp")
            nc.sync.dma_start(out=wp[:], in_=wp_in[:])
            iden = singles.tile([128, 128], F16, name="iden")
            nc.sync.dma_start(out=iden[:], in_=id_in[:])

            qt = [singles.tile([128, T], F16, name=f"qt{m}") for m in range(2)]
            kt = [singles.tile([128, T], F16, name=f"kt{m}") for m in range(2)]
            v_sb = [singles.tile([128, HG * (DH + 1)], F16, name=f"v{tt}")
                    for tt in range(16)]
            for tt in range(16):
                nc.vector.memset(v_sb[tt][:, DH:HG * (DH + 1):DH + 1], 1.0)
            ot = [singles.tile([128, T], F16, name=f"ot{hp}") for hp in range(2)]

            # ---- PE warmup: chew through the pstate ramp while DMAs land
            for i in range(8):
                wps = psmm.tile([128, 512], F32, tag="mm", name=f"warm{i}")
                nc.tensor.matmul(wps[:], lhsT=warm[:, 0:128], rhs=warm[:],
                                 start=True, stop=True)

            # ---- building blocks ---------------------------------------
            def k_block(km, tb):
                """K projection for 512 tokens -> kt[km][:, tb*512:...]

                Two half tiles so the psum->sbuf copy of the first 256
                tokens overlaps the second half's matmuls."""
                for half in range(2):
                    s = 2 * tb + half
                    ps = psmm.tile([128, 256], F32, tag="mm", name=f"k{km}_{s}")
                    for ci in range(8):
                        nc.tensor.matmul(
                            ps[:],
                            lhsT=wk[:, ci * 256 + km * 128: ci * 256 + (km + 1) * 128],
                            rhs=xt3[:, ci, s * 256:(s + 1) * 256],
                            start=(ci == 0),
                            stop=(ci == 7),
                        )
                    nc.vector.tensor_copy(kt[km][:, s * 256:(s + 1) * 256], ps[:])

            def q_block(qm, tb):
                for half in range(2):
                    s = 2 * tb + half
                    ps = psmm.tile([128, 256], F32, tag="mm", name=f"q{qm}_{s}")
                    for ci in range(8):
                        nc.tensor.matmul(
                            ps[:],
                            lhsT=wq[:, ci * 256 + qm * 128: ci * 256 + (qm + 1) * 128],
                            rhs=xt3[:, ci, s * 256:(s + 1) * 256],
                            start=(ci == 0),
                            stop=(ci == 7),
                        )
                    nc.vector.tensor_scalar_add(
                        qt[qm][:, s * 256:(s + 1) * 256], ps[:], bq[:, qm:qm + 1])

            def v_block(tt):
                """V projection for 128 tokens -> v_sb[tt] (65-col head blocks)"""
                ps = psmm.tile([128, 256], F32, tag="mm", name=f"v{tt}")
                for ci in range(8):
                    nc.tensor.matmul(
                        ps[:],
                        lhsT=xt3[:, ci, tt * 128:(tt + 1) * 128],
                        rhs=wv[:, ci * 256:(ci + 1) * 256],
                        start=(ci == 0),
                        stop=(ci == 7),
                    )
                dst = v_sb[tt][:].rearrange("p (h c) -> p h c", h=HG)[:, :, 0:DH]
                src = ps[:].rearrange("p (h c) -> p h c", h=HG)
                nc.vector.tensor_copy(dst, src)

            # scores tile p of unit (h, qb): key chunks 2p,2p+1 x 512 queries
            pt_tiles = {}

            def sc_tile(h, qb, p):
                km = h // 2
                row = (h % 2) * 64
                ps = pssc.tile([128, 1024], F32, tag="sc", name=f"s{h}_{qb}_{p}")
                for half in range(2):
                    st = 2 * p + half
                    nc.tensor.matmul(
                        ps[:, half * 512:(half + 1) * 512],
                        lhsT=kt[km][row:row + 64, st * 128:(st + 1) * 128],
                        rhs=qt[km][row:row + 64, qb * 512:(qb + 1) * 512],
                        start=True,
                        stop=True,
                    )
                pt = ptp.tile([128, 1024], F16, tag="pt", name=f"p{h}_{qb}_{p}")
                nc.scalar.activation(pt[:], ps[:], Exp, scale=SCALE)
                pt_tiles[(h, qb, p)] = pt

            osb_tiles = {}
            # one PSUM bank holds 4 rotating 65-col AV slots
            av_all = psav.tile([128, 4 * (DH + 1)], F32, name="av_all")
            av_ctr = [0]

            def av_group(h, qb, g):
                """AV^T for queries qtile=qb*4+g of head h -> divide into osb."""
                hp, col = h // 2, (h % 2) * 64
                slot = av_ctr[0] % 4
                av_ctr[0] += 1
                av = av_all[:, slot * (DH + 1):(slot + 1) * (DH + 1)]
                for st in range(16):
                    ptk = pt_tiles[(h, qb, st // 2)]
                    nc.tensor.matmul(
                        av[:],
                        lhsT=ptk[:, (st % 2) * 512 + g * 128:
                                 (st % 2) * 512 + (g + 1) * 128],
                        rhs=v_sb[st][:, h * (DH + 1):(h + 1) * (DH + 1)],
                        start=(st == 0),
                        stop=(st == 15),
                    )
                key = (hp, qb, g)
                if key not in osb_tiles:
                    osb_tiles[key] = osbp.tile([128, 128], F16, tag="osb",
                                               name=f"o{hp}_{qb}_{g}")
                rec = osbp.tile([128, 1], F32, tag="rec", bufs=4,
                                name=f"r{h}_{qb}_{g}")
                nc.vector.reciprocal(rec[:], av[:, DH:DH + 1])
                nc.vector.tensor_scalar_mul(
                    osb_tiles[key][:, col:col + 64], av[:, 0:DH], rec[:, 0:1])

            def transpose_hp(hp, qb):
                """osb pair tiles (4 qtiles) -> ot[hp][:, qb*512:...]"""
                ps = psmm.tile([128, 512], F16, tag="mm", name=f"t{hp}_{qb}")
                for g in range(4):
                    nc.tensor.transpose(
                        ps[:, g * 128:(g + 1) * 128],
                        osb_tiles[(hp, qb, g)][:],
                        iden[:],
                    )
                nc.vector.tensor_copy(ot[hp][:, qb * 512:(qb + 1) * 512], ps[:])

            Copy = mybir.ActivationFunctionType.Copy

            def proj_tile(tt, split_dma=False):
                o_out = ooutp.tile([128, C], F16, tag="oout", name=f"oo{tt}")
                for nb in range(2):
                    ps = psmm.tile([128, 512], F32, tag="mm", name=f"pr{tt}_{nb}")
                    for hp in range(2):
                        nc.tensor.matmul(
                            ps[:],
                            lhsT=ot[hp][:, tt * 128:(tt + 1) * 128],
                            rhs=wp[:, hp * C + nb * 512: hp * C + (nb + 1) * 512],
                            start=(hp == 0),
                            stop=(hp == 1),
                        )
                    dst = o_out[:, nb * 512:(nb + 1) * 512]
                    nc.vector.tensor_copy(dst, ps[:])
                    if split_dma:
                        # tail: half-DMAs overlap the copy of the other half
                        nc.sync.dma_start(
                            out=out[tt * 128:(tt + 1) * 128,
                                    nb * 512:(nb + 1) * 512],
                            in_=dst)
                if not split_dma:
                    nc.sync.dma_start(out=out[tt * 128:(tt + 1) * 128, :],
                                      in_=o_out[:])

            # ---- fill queue: transposes + proj consumed in spare PE slots
            fillq = []
            done_av = set()

            def maybe_posts(av_u):
                qb, h = av_u // 4, av_u % 4
                if h == 1:
                    fillq.append(lambda qb=qb: transpose_hp(0, qb))
                elif h == 3 and qb < 3:
                    fillq.append(lambda qb=qb: transpose_hp(1, qb))
                    for tt in range(4 * qb, 4 * qb + 4):
                        fillq.append(lambda tt=tt: proj_tile(tt))

            def full_av(av_u):
                for g in range(4):
                    av_group(av_u % 4, av_u // 4, g)
                done_av.add(av_u)
                maybe_posts(av_u)

            # ---- intro: K + qb0 scores dominate; only 12 V blocks and the
            # qb0 Q blocks live here so ACT stays fed from the start.
            # heads 0,1 share K/Q m-block 0, so their 4 score tiles can all
            # fire right after K0 (+Q0); K1/Q1/V hide under those exps.
            IV = {0: [], 1: [0, 1, 2, 3], 2: [4, 5, 6, 7], 3: [8, 9, 10, 11]}
            for tb in range(4):
                vq = list(IV[tb])
                k_block(0, tb)
                if tb == 0:
                    q_block(0, 0)
                sc_tile(0, 0, 2 * tb)
                sc_tile(0, 0, 2 * tb + 1)
                sc_tile(1, 0, 2 * tb)
                sc_tile(1, 0, 2 * tb + 1)
                k_block(1, tb)
                if tb == 0:
                    q_block(1, 0)
                if vq:
                    v_block(vq.pop(0))
                sc_tile(2, 0, 2 * tb)
                if vq:
                    v_block(vq.pop(0))
                sc_tile(2, 0, 2 * tb + 1)
                if tb == 3:
                    q_block(0, 1)
                sc_tile(3, 0, 2 * tb)
                if vq:
                    v_block(vq.pop(0))
                sc_tile(3, 0, 2 * tb + 1)
                while vq:
                    v_block(vq.pop(0))

            # ---- steady state: units u = qb*4 + h -----------------------
            pre_fills = {}
            mid_fills = {4: [lambda: q_block(1, 1)],
                         5: [lambda: q_block(0, 2)],
                         6: [lambda: q_block(1, 2)],
                         7: [lambda: q_block(0, 3)],
                         8: [lambda: q_block(1, 3)]}
            unit_v = {4: [12, 13], 5: [14, 15]}
            av_plan = {5: [0], 6: [1], 7: [2, 3], 8: [4, 5], 9: [6, 7],
                       10: [8], 11: [9], 12: [10], 13: [11], 14: [12],
                       15: [13, 14]}

            def emit_unit(u):
                qb, h = u // 4, u % 4
                for f in pre_fills.get(u, []):
                    f()
                avs = av_plan.get(u, [])
                first = avs[0] if avs else None
                extras = [lambda tt=tt: v_block(tt) for tt in unit_v.get(u, [])]
                extras += mid_fills.get(u, [])
                nfill = 2
                for p in range(8):
                    sc_tile(h, qb, p)
                    if first is not None and 2 <= p <= 5:
                        g = p - 2
                        av_group(first % 4, first // 4, g)
                        if g == 3:
                            done_av.add(first)
                            maybe_posts(first)
                    elif extras:
                        extras.pop(0)()
                    elif nfill and fillq:
                        nfill -= 1
                        fillq.pop(0)()
                while extras:
                    extras.pop(0)()
                for av_u in avs[1:]:
                    full_av(av_u)

            for u in range(4, 16):
                emit_unit(u)
            # drain: flush pending fills (incl. tp(0,3)), then pipeline the
            # last unit per qtile: AV group -> transpose column -> proj tile.
            while fillq:
                fillq.pop(0)()

            def tp_g(g):
                tps = psmm.tile([128, 128], F16, tag="mm", name=f"tpg{g}")
                nc.tensor.transpose(tps[:], osb_tiles[(1, 3, g)][:], iden[:])
                nc.vector.tensor_copy(
                    ot[1][:, 1536 + g * 128:1536 + (g + 1) * 128], tps[:])

            # interleave so each PE step has >=0.4us of work between
            # cross-engine dependency hops (divide -> transpose -> proj)
            av_group(3, 3, 0)
            av_group(3, 3, 1)
            tp_g(0)
            av_group(3, 3, 2)
            proj_tile(12, split_dma=True)
            tp_g(1)
            av_group(3, 3, 3)
            proj_tile(13, split_dma=True)
            tp_g(2)
            proj_tile(14, split_dma=True)
            tp_g(3)
            proj_tile(15, split_dma=True)

    nc.compile()
    return nc


_CACHE = {}


def _get_nc():
    if "nc" not in _CACHE:
        _CACHE["nc"] = build_bass()
    return _CACHE["nc"]


def _pack8(w):
    """[1024, n] -> [128, 8*n] with [p, ci*n+j] = w[ci*128+p, j]"""
    n = w.shape[1]
    return np.ascontiguousarray(
        w.reshape(8, 128, n).transpose(1, 0, 2).reshape(128, 8 * n))


def make_in_maps(x, w_qkv, b_qkv, w_proj):
    iden = np.eye(128, dtype=np.float16)
    in_maps = []
    for core in range(N_CORES):
        b = core // 4
        hg = core % 4
        cs = slice(hg * DL, (hg + 1) * DL)
        wq = w_qkv[:, 0 * C:1 * C][:, cs].astype(np.float16)
        wk = w_qkv[:, 1 * C:2 * C][:, cs].astype(np.float16)
        wv = w_qkv[:, 2 * C:3 * C][:, cs].astype(np.float16)
        bq = b_qkv[0 * C:1 * C][cs].astype(np.float32)
        xT = np.ascontiguousarray(x[b].T).astype(np.float16)   # [C, T]
        wp2 = w_proj[cs, :].astype(np.float16)                 # [256, 1024]
        wp_pack = np.ascontiguousarray(
            wp2.reshape(2, 128, C).transpose(1, 0, 2).reshape(128, 2 * C))
        in_maps.append({
            "x_pack": _pack8(xT),
            "wk_pack": _pack8(wk),
            "wq_pack": _pack8(wq),
            "wv_pack": _pack8(wv),
            "wp_pack": wp_pack,
            "b_q": np.stack([bq[0:128], bq[128:256]], axis=1),
            "iden16": iden,
        })
    return in_maps


def kernel(x, w_qkv, b_qkv, w_proj, b_proj, **runner_kwargs):
    x = np.asarray(x, dtype=np.float32)
    w_qkv = np.asarray(w_qkv, dtype=np.float32)
    b_qkv = np.asarray(b_qkv, dtype=np.float32)
    w_proj = np.asarray(w_proj, dtype=np.float32)
    b_proj = np.asarray(b_proj, dtype=np.float32)

    nc = _get_nc()
    in_maps = make_in_maps(x, w_qkv, b_qkv, w_proj)
    res = run_bass_kernel_spmd(nc, in_maps, list(range(N_CORES)), **runner_kwargs)
    parts = [res.results[i]["out_partial"] for i in range(N_CORES)]
    # fold V bias through the projection; K bias is softmax-invariant
    b_eff = b_proj + b_qkv[2 * C:3 * C].astype(np.float64) @ w_proj.astype(np.float64)
    outv = np.zeros((B, T, C), dtype=np.float32)
    for b in range(B):
        for hg in range(4):
            outv[b] += parts[4 * b + hg].astype(np.float32)
        outv[b] += b_eff.astype(np.float32)[None, :]
    if runner_kwargs:
        return outv, res
    return outv


if __name__ == "__main__":
    import reference

    inputs = reference.setup_inputs()
    inputs = {k: np.asarray(v) for k, v in inputs.items()}
    got = kernel(**inputs)
    want = np.asarray(reference.reference(**inputs))
    err = np.abs(got - want).max() / np.abs(want).max()
    print("rel err:", err)


# revision 21
# speedup vs baseline: 1.0138x; 1.0138x over previous
"""Multi-head self-attention Trainium2 kernel, sharded over 8 NeuronCores.

Sharding: core = (batch, head_group): 2 batches x 4 head-groups (4 heads each).
Each core computes qkv for its batch restricted to its heads, full-sequence
attention for those heads, and a row-parallel slice of the output projection,
producing a partial [T, C] output (fp16). Host: out[b] = sum of the 4
head-group partials + b_eff where b_eff folds b_proj and the V bias.

v2 design notes (all relative to the fp32/on-chip-transpose baseline):
  - x is transposed, packed and cast to fp16 on the host; no on-chip
    transposes or x^T copies are needed.
  - K bias is dropped entirely (softmax is invariant to per-query constants,
    and q.bk is per-query); V bias is folded into b_proj on the host
    (sum_s w_s = 1); only the Q bias is applied on-chip.
  - AV is computed transposed: out[q, d] = sum_s P[s,q] V[s,d] with
    ap_size=65 per chunk matmul, which halves the PE cost of AV and makes
    the softmax divide a single per-partition tensor_scalar divide.
  - The softmax denominator comes from a ones-column appended per head in
    the V tile (memset once).
  - Everything on the PE runs fp16 (1.0 cycles/row); fp8 was measured to
    break the 2e-2 gate (diffuse attention preserves per-key noise).
"""

import math
import sys

import numpy as np

sys.path.insert(0, "/opt/trn_rl_repo")

import concourse.bacc as bacc
import concourse.bass as bass
import concourse.tile as tile
from concourse import mybir
from concourse.bass_utils import run_bass_kernel_spmd

B, T, C = 2, 2048, 1024
NH, DH = 16, 64
HG = 4                  # heads per core
DL = HG * DH            # 256 local head dims
N_CORES = 8

F32 = mybir.dt.float32
F16 = mybir.dt.float16

SCALE = 1.0 / math.sqrt(DH)
Exp = mybir.ActivationFunctionType.Exp


def build_bass():
    nc = bacc.Bacc("TRN2", target_bir_lowering=False, debug=False)

    # host-packed params: [p, ci*w + j] = w[ci*128 + p, j]
    x_in = nc.declare_dram_parameter("x_pack", [128, 8 * T], F16, isOutput=False)
    wk_in = nc.declare_dram_parameter("wk_pack", [128, 8 * DL], F16, isOutput=False)
    wq_in = nc.declare_dram_parameter("wq_pack", [128, 8 * DL], F16, isOutput=False)
    wv_in = nc.declare_dram_parameter("wv_pack", [128, 8 * DL], F16, isOutput=False)
    wp_in = nc.declare_dram_parameter("wp_pack", [128, 2 * C], F16, isOutput=False)
    bq_in = nc.declare_dram_parameter("b_q", [128, 2], F32, isOutput=False)
    id_in = nc.declare_dram_parameter("iden16", [128, 128], F16, isOutput=False)
    out = nc.declare_dram_parameter("out_partial", [T, C], F16, isOutput=True)

    with tile.TileContext(nc) as tc:
        with (
            tc.tile_pool(name="singles", bufs=1) as singles,
            tc.tile_pool(name="pt", bufs=44) as ptp,
            tc.tile_pool(name="osb", bufs=6) as osbp,
            tc.tile_pool(name="oout", bufs=3) as ooutp,
            tc.tile_pool(name="sc", bufs=2, space="PSUM") as pssc,     # 2x2 banks
            tc.tile_pool(name="avp", bufs=1, space="PSUM") as psav,    # 1 bank
            tc.tile_pool(name="mm", bufs=3, space="PSUM") as psmm,     # 3x1 bank
        ):
            # ---- persistent sbuf tiles ---------------------------------
            warm = singles.tile([128, 512], F16, name="warm")
            nc.vector.memset(warm[:], 0.0)
            # pre-load the Exp activation table while DMAs are in flight
            warm_exp = singles.tile([128, 1], F16, name="warm_exp")
            nc.scalar.activation(warm_exp[:], warm[:, 0:1], Exp, scale=SCALE)

            xt = singles.tile([128, 8 * T], F16, name="xt")
            xt3 = xt[:].rearrange("p (ci t) -> p ci t", ci=8)
            xsrc = x_in[:].rearrange("p (ci t) -> p ci t", ci=8)
            NSL = 8
            TSL = T // NSL

            def x_slice(s):
                nc.sync.dma_start(
                    out=xt3[:, :, s * TSL:(s + 1) * TSL],
                    in_=xsrc[:, :, s * TSL:(s + 1) * TSL],
                )

            # DMA order tuned so the first score tile unblocks earliest:
            # Q projection (wq + x s0,s1) is the long pole for score p0.
            wq = singles.tile([128, 8 * DL], F16, name="wq")
            nc.sync.dma_start(out=wq[:], in_=wq_in[:])
            bq = singles.tile([128, 2], F32, name="bq")
            nc.sync.dma_start(out=bq[:], in_=bq_in[:])
            x_slice(0)
            x_slice(1)
            wk = singles.tile([128, 8 * DL], F16, name="wk")
            nc.sync.dma_start(out=wk[:], in_=wk_in[:])
            x_slice(2)
            x_slice(3)
            wv = singles.tile([128, 8 * DL], F16, name="wv")
            nc.sync.dma_start(out=wv[:], in_=wv_in[:])
            x_slice(4)
            x_slice(5)
            x_slice(6)
            x_slice(7)
            wp = singles.tile([128, 2 * C], F16, name="wp")
            nc.sync.dma_start(out=wp[:], in_=wp_in[:])
            iden = singles.tile([128, 128], F16, name="iden")
            nc.sync.dma_start(out=iden[:], in_=id_in[:])

            qt = [singles.tile([128, T], F16, name=f"qt{m}") for m in range(2)]
            kt = [singles.tile([128, T], F16, name=f"kt{m}") for m in range(2)]
            v_sb = [singles.tile([128, HG * (DH + 1)], F16, name=f"v{tt}")
                    for tt in range(16)]
            for tt in range(16):
                nc.vector.memset(v_sb[tt][:, DH:HG * (DH + 1):DH + 1], 1.0)
            ot = [singles.tile([128, T], F16, name=f"ot{hp}") for hp in range(2)]

            # ---- PE warmup: chew through the pstate ramp while DMAs land
            for i in range(8):
                wps = psmm.tile([128, 512], F32, tag="mm", name=f"warm{i}")
                nc.tensor.matmul(wps[:], lhsT=warm[:, 0:128], rhs=warm[:],
                                 start=True, stop=True)

            # ---- building blocks ---------------------------------------
            def k_block(km, tb):
                """K projection for 512 tokens -> kt[km][:, tb*512:...]

                Two half tiles so the psum->sbuf copy of the first 256
                tokens overlaps the second half's matmuls."""
                for half in range(2):
                    s = 2 * tb + half
                    ps = psmm.tile([128, 256], F32, tag="mm", name=f"k{km}_{s}")
                    for ci in range(8):
                        nc.tensor.matmul(
                            ps[:],
                            lhsT=wk[:, ci * 256 + km * 128: ci * 256 + (km + 1) * 128],
                            rhs=xt3[:, ci, s * 256:(s + 1) * 256],
                            start=(ci == 0),
                            stop=(ci == 7),
                        )
                    nc.vector.tensor_copy(kt[km][:, s * 256:(s + 1) * 256], ps[:])

            def q_block(qm, tb):
                for half in range(2):
                    s = 2 * tb + half
                    ps = psmm.tile([128, 256], F32, tag="mm", name=f"q{qm}_{s}")
                    for ci in range(8):
                        nc.tensor.matmul(
                            ps[:],
                            lhsT=wq[:, ci * 256 + qm * 128: ci * 256 + (qm + 1) * 128],
                            rhs=xt3[:, ci, s * 256:(s + 1) * 256],
                            start=(ci == 0),
                            stop=(ci == 7),
                        )
                    nc.vector.tensor_scalar_add(
                        qt[qm][:, s * 256:(s + 1) * 256], ps[:], bq[:, qm:qm + 1])

            def v_block(tt):
                """V projection for 128 tokens -> v_sb[tt] (65-col head blocks)"""
                ps = psmm.tile([128, 256], F32, tag="mm", name=f"v{tt}")
                for ci in range(8):
                    nc.tensor.matmul(
                        ps[:],
                        lhsT=xt3[:, ci, tt * 128:(tt + 1) * 128],
                        rhs=wv[:, ci * 256:(ci + 1) * 256],
                        start=(ci == 0),
                        stop=(ci == 7),
                    )
                dst = v_sb[tt][:].rearrange("p (h c) -> p h c", h=HG)[:, :, 0:DH]
                src = ps[:].rearrange("p (h c) -> p h c", h=HG)
                nc.vector.tensor_copy(dst, src)

            # scores tile p of unit (h, qb): key chunks 2p,2p+1 x 512 queries
            pt_tiles = {}

            def sc_tile(h, qb, p):
                km = h // 2
                row = (h % 2) * 64
                ps = pssc.tile([128, 1024], F32, tag="sc", name=f"s{h}_{qb}_{p}")
                for half in range(2):
                    st = 2 * p + half
                    nc.tensor.matmul(
                        ps[:, half * 512:(half + 1) * 512],
                        lhsT=kt[km][row:row + 64, st * 128:(st + 1) * 128],
                        rhs=qt[km][row:row + 64, qb * 512:(qb + 1) * 512],
                        start=True,
                        stop=True,
                    )
                pt = ptp.tile([128, 1024], F16, tag="pt", name=f"p{h}_{qb}_{p}")
                nc.scalar.activation(pt[:], ps[:], Exp, scale=SCALE)
                pt_tiles[(h, qb, p)] = pt

            osb_tiles = {}
            # one PSUM bank holds 4 rotating 65-col AV slots
            av_all = psav.tile([128, 4 * (DH + 1)], F32, name="av_all")
            av_ctr = [0]

            def av_group(h, qb, g):
                """AV^T for queries qtile=qb*4+g of head h -> divide into osb."""
                hp, col = h // 2, (h % 2) * 64
                slot = av_ctr[0] % 4
                av_ctr[0] += 1
                av = av_all[:, slot * (DH + 1):(slot + 1) * (DH + 1)]
                for st in range(16):
                    ptk = pt_tiles[(h, qb, st // 2)]
                    nc.tensor.matmul(
                        av[:],
                        lhsT=ptk[:, (st % 2) * 512 + g * 128:
                                 (st % 2) * 512 + (g + 1) * 128],
                        rhs=v_sb[st][:, h * (DH + 1):(h + 1) * (DH + 1)],
                        start=(st == 0),
                        stop=(st == 15),
                    )
                key = (hp, qb, g)
                if key not in osb_tiles:
                    osb_tiles[key] = osbp.tile([128, 128], F16, tag="osb",
                                               name=f"o{hp}_{qb}_{g}")
                rec = osbp.tile([128, 1], F32, tag="rec", bufs=4,
                                name=f"r{h}_{qb}_{g}")
                nc.vector.reciprocal(rec[:], av[:, DH:DH + 1])
                nc.vector.tensor_scalar_mul(
                    osb_tiles[key][:, col:col + 64], av[:, 0:DH], rec[:, 0:1])

            def transpose_hp(hp, qb):
                """osb pair tiles (4 qtiles) -> ot[hp][:, qb*512:...]"""
                ps = psmm.tile([128, 512], F16, tag="mm", name=f"t{hp}_{qb}")
                for g in range(4):
                    nc.tensor.transpose(
                        ps[:, g * 128:(g + 1) * 128],
                        osb_tiles[(hp, qb, g)][:],
                        iden[:],
                    )
                nc.vector.tensor_copy(ot[hp][:, qb * 512:(qb + 1) * 512], ps[:])

            Copy = mybir.ActivationFunctionType.Copy

            def proj_tile(tt, use_act=False):
                o_out = ooutp.tile([128, C], F16, tag="oout", name=f"oo{tt}")
                for nb in range(2):
                    ps = psmm.tile([128, 512], F32, tag="mm", name=f"pr{tt}_{nb}")
                    for hp in range(2):
                        nc.tensor.matmul(
                            ps[:],
                            lhsT=ot[hp][:, tt * 128:(tt + 1) * 128],
                            rhs=wp[:, hp * C + nb * 512: hp * C + (nb + 1) * 512],
                            start=(hp == 0),
                            stop=(hp == 1),
                        )
                    dst = o_out[:, nb * 512:(nb + 1) * 512]
                    if use_act:
                        # tail: ACT is idle after the last exp, DVE is not;
                        # half-DMAs overlap the copy of the other half
                        nc.scalar.activation(dst, ps[:], Copy)
                        nc.sync.dma_start(
                            out=out[tt * 128:(tt + 1) * 128,
                                    nb * 512:(nb + 1) * 512],
                            in_=dst)
                    else:
                        nc.vector.tensor_copy(dst, ps[:])
                if not use_act:
                    nc.sync.dma_start(out=out[tt * 128:(tt + 1) * 128, :],
                                      in_=o_out[:])

            # ---- fill queue: transposes + proj consumed in spare PE slots
            fillq = []
            done_av = set()

            def maybe_posts(av_u):
                qb, h = av_u // 4, av_u % 4
                if h == 1:
                    fillq.append(lambda qb=qb: transpose_hp(0, qb))
                elif h == 3 and qb < 3:
                    fillq.append(lambda qb=qb: transpose_hp(1, qb))
                    for tt in range(4 * qb, 4 * qb + 4):
                        fillq.append(lambda tt=tt: proj_tile(tt))

            def full_av(av_u):
                for g in range(4):
                    av_group(av_u % 4, av_u // 4, g)
                done_av.add(av_u)
                maybe_posts(av_u)

            # ---- intro: K + qb0 scores dominate; only 12 V blocks and the
            # qb0 Q blocks live here so ACT stays fed from the start.
            # heads 0,1 share K/Q m-block 0, so their 4 score tiles can all
            # fire right after K0 (+Q0); K1/Q1/V hide under those exps.
            IV = {0: [0], 1: [1, 2, 3], 2: [4, 5, 6], 3: [7, 8, 9, 10, 11]}
            for tb in range(4):
                vq = list(IV[tb])
                if tb == 0:
                    q_block(0, 0)
                k_block(0, tb)
                sc_tile(0, 0, 2 * tb)
                sc_tile(0, 0, 2 * tb + 1)
                sc_tile(1, 0, 2 * tb)
                sc_tile(1, 0, 2 * tb + 1)
                k_block(1, tb)
                if tb == 0:
                    q_block(1, 0)
                if vq:
                    v_block(vq.pop(0))
                sc_tile(2, 0, 2 * tb)
                if vq:
                    v_block(vq.pop(0))
                sc_tile(2, 0, 2 * tb + 1)
                sc_tile(3, 0, 2 * tb)
                if vq:
                    v_block(vq.pop(0))
                sc_tile(3, 0, 2 * tb + 1)
                while vq:
                    v_block(vq.pop(0))

            # ---- steady state: units u = qb*4 + h -----------------------
            pre_fills = {4: [lambda: q_block(0, 1)]}
            mid_fills = {5: [lambda: q_block(1, 1)],
                         6: [lambda: q_block(0, 2)],
                         7: [lambda: q_block(1, 2)],
                         8: [lambda: q_block(0, 3)],
                         9: [lambda: q_block(1, 3)]}
            unit_v = {4: [12, 13, 14, 15]}
            av_plan = {5: [0], 6: [1], 7: [2, 3], 8: [4, 5], 9: [6, 7],
                       10: [8], 11: [9], 12: [10], 13: [11], 14: [12],
                       15: [13, 14]}

            def emit_unit(u):
                qb, h = u // 4, u % 4
                for f in pre_fills.get(u, []):
                    f()
                avs = av_plan.get(u, [])
                first = avs[0] if avs else None
                extras = [lambda tt=tt: v_block(tt) for tt in unit_v.get(u, [])]
                extras += mid_fills.get(u, [])
                nfill = 2
                for p in range(8):
                    sc_tile(h, qb, p)
                    if first is not None and 2 <= p <= 5:
                        g = p - 2
                        av_group(first % 4, first // 4, g)
                        if g == 3:
                            done_av.add(first)
                            maybe_posts(first)
                    elif extras:
                        extras.pop(0)()
                    elif nfill and fillq:
                        nfill -= 1
                        fillq.pop(0)()
                while extras:
                    extras.pop(0)()
                for av_u in avs[1:]:
                    full_av(av_u)

            for u in range(4, 16):
                emit_unit(u)
            # drain: flush pending fills (incl. tp(0,3)), then pipeline the
            # last unit per qtile: AV group -> transpose column -> proj tile.
            while fillq:
                fillq.pop(0)()

            for g in range(4):
                av_group(3, 3, g)
                tps = psmm.tile([128, 128], F16, tag="mm", name=f"tpg{g}")
                nc.tensor.transpose(tps[:], osb_tiles[(1, 3, g)][:], iden[:])
                nc.vector.tensor_copy(
                    ot[1][:, 1536 + g * 128:1536 + (g + 1) * 128], tps[:])
                proj_tile(12 + g, use_act=True)

    nc.compile()
    return nc


_CACHE = {}


def _get_nc():
    if "nc" not in _CACHE:
        _CACHE["nc"] = build_bass()
    return _CACHE["nc"]


def _pack8(w):
    """[1024, n] -> [128, 8*n] with [p, ci*n+j] = w[ci*128+p, j]"""
    n = w.shape[1]
    return np.ascontiguousarray(
        w.reshape(8, 128, n).transpose(1, 0, 2).reshape(128, 8 * n))


def make_in_maps(x, w_qkv, b_qkv, w_proj):
    iden = np.eye(128, dtype=np.float16)
    in_maps = []
    for core in range(N_CORES):
        b = core // 4
        hg = core % 4
        cs = slice(hg * DL, (hg + 1) * DL)
        wq = w_qkv[:, 0 * C:1 * C][:, cs].astype(np.float16)
        wk = w_qkv[:, 1 * C:2 * C][:, cs].astype(np.float16)
        wv = w_qkv[:, 2 * C:3 * C][:, cs].astype(np.float16)
        bq = b_qkv[0 * C:1 * C][cs].astype(np.float32)
        xT = np.ascontiguousarray(x[b].T).astype(np.float16)   # [C, T]
        wp2 = w_proj[cs, :].astype(np.float16)                 # [256, 1024]
        wp_pack = np.ascontiguousarray(
            wp2.reshape(2, 128, C).transpose(1, 0, 2).reshape(128, 2 * C))
        in_maps.append({
            "x_pack": _pack8(xT),
            "wk_pack": _pack8(wk),
            "wq_pack": _pack8(wq),
            "wv_pack": _pack8(wv),
            "wp_pack": wp_pack,
            "b_q": np.stack([bq[0:128], bq[128:256]], axis=1),
            "iden16": iden,
        })
    return in_maps


def kernel(x, w_qkv, b_qkv, w_proj, b_proj, **runner_kwargs):
    x = np.asarray(x, dtype=np.float32)
    w_qkv = np.asarray(w_qkv, dtype=np.float32)
    b_qkv = np.asarray(b_qkv, dtype=np.float32)
    w_proj = np.asarray(w_proj, dtype=np.float32)
    b_proj = np.asarray(b_proj, dtype=np.float32)

    nc = _get_nc()
    in_maps = make_in_maps(x, w_qkv, b_qkv, w_proj)
    res = run_bass_kernel_spmd(nc, in_maps, list(range(N_CORES)), **runner_kwargs)
    parts = [res.results[i]["out_partial"] for i in range(N_CORES)]
    # fold V bias through the projection; K bias is softmax-invariant
    b_eff = b_proj + b_qkv[2 * C:3 * C].astype(np.float64) @ w_proj.astype(np.float64)
    outv = np.zeros((B, T, C), dtype=np.float32)
    for b in range(B):
        for hg in range(4):
            outv[b] += parts[4 * b + hg].astype(np.float32)
        outv[b] += b_eff.astype(np.float32)[None, :]
    if runner_kwargs:
        return outv, res
    return outv


if __name__ == "__main__":
    import reference

    inputs = reference.setup_inputs()
    inputs = {k: np.asarray(v) for k, v in inputs.items()}
    got = kernel(**inputs)
    want = np.asarray(reference.reference(**inputs))
    err = np.abs(got - want).max() / np.abs(want).max()
    print("rel err:", err)


# revision 22
# speedup vs baseline: 1.0208x; 1.0069x over previous
"""Multi-head self-attention Trainium2 kernel, sharded over 8 NeuronCores.

Sharding: core = (batch, head_group): 2 batches x 4 head-groups (4 heads each).
Each core computes qkv for its batch restricted to its heads, full-sequence
attention for those heads, and a row-parallel slice of the output projection,
producing a partial [T, C] output (fp16). Host: out[b] = sum of the 4
head-group partials + b_eff where b_eff folds b_proj and the V bias.

v2 design notes (all relative to the fp32/on-chip-transpose baseline):
  - x is transposed, packed and cast to fp16 on the host; no on-chip
    transposes or x^T copies are needed.
  - K bias is dropped entirely (softmax is invariant to per-query constants,
    and q.bk is per-query); V bias is folded into b_proj on the host
    (sum_s w_s = 1); only the Q bias is applied on-chip.
  - AV is computed transposed: out[q, d] = sum_s P[s,q] V[s,d] with
    ap_size=65 per chunk matmul, which halves the PE cost of AV and makes
    the softmax divide a single per-partition tensor_scalar divide.
  - The softmax denominator comes from a ones-column appended per head in
    the V tile (memset once).
  - Everything on the PE runs fp16 (1.0 cycles/row); fp8 was measured to
    break the 2e-2 gate (diffuse attention preserves per-key noise).
"""

import math
import sys

import numpy as np

sys.path.insert(0, "/opt/trn_rl_repo")

import concourse.bacc as bacc
import concourse.bass as bass
import concourse.tile as tile
from concourse import mybir
from concourse.bass_utils import run_bass_kernel_spmd

B, T, C = 2, 2048, 1024
NH, DH = 16, 64
HG = 4                  # heads per core
DL = HG * DH            # 256 local head dims
N_CORES = 8

F32 = mybir.dt.float32
F16 = mybir.dt.float16

SCALE = 1.0 / math.sqrt(DH)
Exp = mybir.ActivationFunctionType.Exp


def build_bass():
    nc = bacc.Bacc("TRN2", target_bir_lowering=False, debug=False)

    # host-packed params: [p, ci*w + j] = w[ci*128 + p, j]
    x_in = nc.declare_dram_parameter("x_pack", [128, 8 * T], F16, isOutput=False)
    wk_in = nc.declare_dram_parameter("wk_pack", [128, 8 * DL], F16, isOutput=False)
    wq_in = nc.declare_dram_parameter("wq_pack", [128, 8 * DL], F16, isOutput=False)
    wv_in = nc.declare_dram_parameter("wv_pack", [128, 8 * DL], F16, isOutput=False)
    wp_in = nc.declare_dram_parameter("wp_pack", [128, 2 * C], F16, isOutput=False)
    bq_in = nc.declare_dram_parameter("b_q", [128, 2], F32, isOutput=False)
    id_in = nc.declare_dram_parameter("iden16", [128, 128], F16, isOutput=False)
    out = nc.declare_dram_parameter("out_partial", [T, C], F16, isOutput=True)

    with tile.TileContext(nc) as tc:
        with (
            tc.tile_pool(name="singles", bufs=1) as singles,
            tc.tile_pool(name="pt", bufs=44) as ptp,
            tc.tile_pool(name="osb", bufs=6) as osbp,
            tc.tile_pool(name="oout", bufs=3) as ooutp,
            tc.tile_pool(name="sc", bufs=2, space="PSUM") as pssc,     # 2x2 banks
            tc.tile_pool(name="avp", bufs=1, space="PSUM") as psav,    # 1 bank
            tc.tile_pool(name="mm", bufs=3, space="PSUM") as psmm,     # 3x1 bank
        ):
            # ---- persistent sbuf tiles ---------------------------------
            warm = singles.tile([128, 512], F16, name="warm")
            nc.vector.memset(warm[:], 0.0)
            # pre-load the Exp activation table while DMAs are in flight
            warm_exp = singles.tile([128, 1], F16, name="warm_exp")
            nc.scalar.activation(warm_exp[:], warm[:, 0:1], Exp, scale=SCALE)

            xt = singles.tile([128, 8 * T], F16, name="xt")
            xt3 = xt[:].rearrange("p (ci t) -> p ci t", ci=8)
            xsrc = x_in[:].rearrange("p (ci t) -> p ci t", ci=8)
            NSL = 8
            TSL = T // NSL

            def x_slice(s):
                nc.sync.dma_start(
                    out=xt3[:, :, s * TSL:(s + 1) * TSL],
                    in_=xsrc[:, :, s * TSL:(s + 1) * TSL],
                )

            # DMA order tuned so the first score tile unblocks earliest:
            # Q projection (wq + x s0,s1) is the long pole for score p0.
            wq = singles.tile([128, 8 * DL], F16, name="wq")
            nc.sync.dma_start(out=wq[:], in_=wq_in[:])
            bq = singles.tile([128, 2], F32, name="bq")
            nc.sync.dma_start(out=bq[:], in_=bq_in[:])
            x_slice(0)
            x_slice(1)
            wk = singles.tile([128, 8 * DL], F16, name="wk")
            nc.sync.dma_start(out=wk[:], in_=wk_in[:])
            x_slice(2)
            x_slice(3)
            wv = singles.tile([128, 8 * DL], F16, name="wv")
            nc.sync.dma_start(out=wv[:], in_=wv_in[:])
            x_slice(4)
            x_slice(5)
            x_slice(6)
            x_slice(7)
            wp = singles.tile([128, 2 * C], F16, name="wp")
            nc.sync.dma_start(out=wp[:], in_=wp_in[:])
            iden = singles.tile([128, 128], F16, name="iden")
            nc.sync.dma_start(out=iden[:], in_=id_in[:])

            qt = [singles.tile([128, T], F16, name=f"qt{m}") for m in range(2)]
            kt = [singles.tile([128, T], F16, name=f"kt{m}") for m in range(2)]
            v_sb = [singles.tile([128, HG * (DH + 1)], F16, name=f"v{tt}")
                    for tt in range(16)]
            for tt in range(16):
                nc.vector.memset(v_sb[tt][:, DH:HG * (DH + 1):DH + 1], 1.0)
            ot = [singles.tile([128, T], F16, name=f"ot{hp}") for hp in range(2)]

            # ---- PE warmup: chew through the pstate ramp while DMAs land
            for i in range(8):
                wps = psmm.tile([128, 512], F32, tag="mm", name=f"warm{i}")
                nc.tensor.matmul(wps[:], lhsT=warm[:, 0:128], rhs=warm[:],
                                 start=True, stop=True)

            # ---- building blocks ---------------------------------------
            def k_block(km, tb):
                """K projection for 512 tokens -> kt[km][:, tb*512:...]

                Two half tiles so the psum->sbuf copy of the first 256
                tokens overlaps the second half's matmuls."""
                for half in range(2):
                    s = 2 * tb + half
                    ps = psmm.tile([128, 256], F32, tag="mm", name=f"k{km}_{s}")
                    for ci in range(8):
                        nc.tensor.matmul(
                            ps[:],
                            lhsT=wk[:, ci * 256 + km * 128: ci * 256 + (km + 1) * 128],
                            rhs=xt3[:, ci, s * 256:(s + 1) * 256],
                            start=(ci == 0),
                            stop=(ci == 7),
                        )
                    nc.vector.tensor_copy(kt[km][:, s * 256:(s + 1) * 256], ps[:])

            def q_block(qm, tb):
                for half in range(2):
                    s = 2 * tb + half
                    ps = psmm.tile([128, 256], F32, tag="mm", name=f"q{qm}_{s}")
                    for ci in range(8):
                        nc.tensor.matmul(
                            ps[:],
                            lhsT=wq[:, ci * 256 + qm * 128: ci * 256 + (qm + 1) * 128],
                            rhs=xt3[:, ci, s * 256:(s + 1) * 256],
                            start=(ci == 0),
                            stop=(ci == 7),
                        )
                    nc.vector.tensor_scalar_add(
                        qt[qm][:, s * 256:(s + 1) * 256], ps[:], bq[:, qm:qm + 1])

            def v_block(tt):
                """V projection for 128 tokens -> v_sb[tt] (65-col head blocks)"""
                ps = psmm.tile([128, 256], F32, tag="mm", name=f"v{tt}")
                for ci in range(8):
                    nc.tensor.matmul(
                        ps[:],
                        lhsT=xt3[:, ci, tt * 128:(tt + 1) * 128],
                        rhs=wv[:, ci * 256:(ci + 1) * 256],
                        start=(ci == 0),
                        stop=(ci == 7),
                    )
                dst = v_sb[tt][:].rearrange("p (h c) -> p h c", h=HG)[:, :, 0:DH]
                src = ps[:].rearrange("p (h c) -> p h c", h=HG)
                nc.vector.tensor_copy(dst, src)

            # scores tile p of unit (h, qb): key chunks 2p,2p+1 x 512 queries
            pt_tiles = {}

            def sc_tile(h, qb, p):
                km = h // 2
                row = (h % 2) * 64
                ps = pssc.tile([128, 1024], F32, tag="sc", name=f"s{h}_{qb}_{p}")
                for half in range(2):
                    st = 2 * p + half
                    nc.tensor.matmul(
                        ps[:, half * 512:(half + 1) * 512],
                        lhsT=kt[km][row:row + 64, st * 128:(st + 1) * 128],
                        rhs=qt[km][row:row + 64, qb * 512:(qb + 1) * 512],
                        start=True,
                        stop=True,
                    )
                pt = ptp.tile([128, 1024], F16, tag="pt", name=f"p{h}_{qb}_{p}")
                nc.scalar.activation(pt[:], ps[:], Exp, scale=SCALE)
                pt_tiles[(h, qb, p)] = pt

            osb_tiles = {}
            # one PSUM bank holds 4 rotating 65-col AV slots
            av_all = psav.tile([128, 4 * (DH + 1)], F32, name="av_all")
            av_ctr = [0]

            def av_group(h, qb, g):
                """AV^T for queries qtile=qb*4+g of head h -> divide into osb."""
                hp, col = h // 2, (h % 2) * 64
                slot = av_ctr[0] % 4
                av_ctr[0] += 1
                av = av_all[:, slot * (DH + 1):(slot + 1) * (DH + 1)]
                for st in range(16):
                    ptk = pt_tiles[(h, qb, st // 2)]
                    nc.tensor.matmul(
                        av[:],
                        lhsT=ptk[:, (st % 2) * 512 + g * 128:
                                 (st % 2) * 512 + (g + 1) * 128],
                        rhs=v_sb[st][:, h * (DH + 1):(h + 1) * (DH + 1)],
                        start=(st == 0),
                        stop=(st == 15),
                    )
                key = (hp, qb, g)
                if key not in osb_tiles:
                    osb_tiles[key] = osbp.tile([128, 128], F16, tag="osb",
                                               name=f"o{hp}_{qb}_{g}")
                rec = osbp.tile([128, 1], F32, tag="rec", bufs=4,
                                name=f"r{h}_{qb}_{g}")
                nc.vector.reciprocal(rec[:], av[:, DH:DH + 1])
                nc.vector.tensor_scalar_mul(
                    osb_tiles[key][:, col:col + 64], av[:, 0:DH], rec[:, 0:1])

            def transpose_hp(hp, qb):
                """osb pair tiles (4 qtiles) -> ot[hp][:, qb*512:...]"""
                ps = psmm.tile([128, 512], F16, tag="mm", name=f"t{hp}_{qb}")
                for g in range(4):
                    nc.tensor.transpose(
                        ps[:, g * 128:(g + 1) * 128],
                        osb_tiles[(hp, qb, g)][:],
                        iden[:],
                    )
                nc.vector.tensor_copy(ot[hp][:, qb * 512:(qb + 1) * 512], ps[:])

            Copy = mybir.ActivationFunctionType.Copy

            def proj_tile(tt, use_act=False):
                o_out = ooutp.tile([128, C], F16, tag="oout", name=f"oo{tt}")
                for nb in range(2):
                    ps = psmm.tile([128, 512], F32, tag="mm", name=f"pr{tt}_{nb}")
                    for hp in range(2):
                        nc.tensor.matmul(
                            ps[:],
                            lhsT=ot[hp][:, tt * 128:(tt + 1) * 128],
                            rhs=wp[:, hp * C + nb * 512: hp * C + (nb + 1) * 512],
                            start=(hp == 0),
                            stop=(hp == 1),
                        )
                    dst = o_out[:, nb * 512:(nb + 1) * 512]
                    if use_act:
                        # tail: ACT is idle after the last exp, DVE is not;
                        # half-DMAs overlap the copy of the other half
                        nc.scalar.activation(dst, ps[:], Copy)
                        nc.sync.dma_start(
                            out=out[tt * 128:(tt + 1) * 128,
                                    nb * 512:(nb + 1) * 512],
                            in_=dst)
                    else:
                        nc.vector.tensor_copy(dst, ps[:])
                if not use_act:
                    nc.sync.dma_start(out=out[tt * 128:(tt + 1) * 128, :],
                                      in_=o_out[:])

            # ---- fill queue: transposes + proj consumed in spare PE slots
            fillq = []
            done_av = set()

            def maybe_posts(av_u):
                qb, h = av_u // 4, av_u % 4
                if h == 1:
                    fillq.append(lambda qb=qb: transpose_hp(0, qb))
                elif h == 3 and qb < 3:
                    fillq.append(lambda qb=qb: transpose_hp(1, qb))
                    for tt in range(4 * qb, 4 * qb + 4):
                        fillq.append(lambda tt=tt: proj_tile(tt))

            def full_av(av_u):
                for g in range(4):
                    av_group(av_u % 4, av_u // 4, g)
                done_av.add(av_u)
                maybe_posts(av_u)

            # ---- intro: K + qb0 scores dominate; only 12 V blocks and the
            # qb0 Q blocks live here so ACT stays fed from the start.
            # heads 0,1 share K/Q m-block 0, so their 4 score tiles can all
            # fire right after K0 (+Q0); K1/Q1/V hide under those exps.
            IV = {0: [0], 1: [1, 2, 3], 2: [4, 5, 6], 3: [7, 8, 9, 10, 11]}
            for tb in range(4):
                vq = list(IV[tb])
                if tb == 0:
                    q_block(0, 0)
                k_block(0, tb)
                sc_tile(0, 0, 2 * tb)
                sc_tile(0, 0, 2 * tb + 1)
                sc_tile(1, 0, 2 * tb)
                sc_tile(1, 0, 2 * tb + 1)
                k_block(1, tb)
                if tb == 0:
                    q_block(1, 0)
                if vq:
                    v_block(vq.pop(0))
                sc_tile(2, 0, 2 * tb)
                if vq:
                    v_block(vq.pop(0))
                sc_tile(2, 0, 2 * tb + 1)
                sc_tile(3, 0, 2 * tb)
                if vq:
                    v_block(vq.pop(0))
                sc_tile(3, 0, 2 * tb + 1)
                while vq:
                    v_block(vq.pop(0))

            # ---- steady state: units u = qb*4 + h -----------------------
            pre_fills = {4: [lambda: q_block(0, 1)]}
            mid_fills = {5: [lambda: q_block(1, 1)],
                         6: [lambda: q_block(0, 2)],
                         7: [lambda: q_block(1, 2)],
                         8: [lambda: q_block(0, 3)],
                         9: [lambda: q_block(1, 3)]}
            unit_v = {4: [12, 13, 14, 15]}
            av_plan = {5: [0], 6: [1], 7: [2, 3], 8: [4, 5], 9: [6, 7],
                       10: [8], 11: [9], 12: [10], 13: [11], 14: [12],
                       15: [13, 14]}

            def emit_unit(u):
                qb, h = u // 4, u % 4
                for f in pre_fills.get(u, []):
                    f()
                avs = av_plan.get(u, [])
                first = avs[0] if avs else None
                extras = [lambda tt=tt: v_block(tt) for tt in unit_v.get(u, [])]
                extras += mid_fills.get(u, [])
                nfill = 2
                for p in range(8):
                    sc_tile(h, qb, p)
                    if first is not None and 2 <= p <= 5:
                        g = p - 2
                        av_group(first % 4, first // 4, g)
                        if g == 3:
                            done_av.add(first)
                            maybe_posts(first)
                    elif extras:
                        extras.pop(0)()
                    elif nfill and fillq:
                        nfill -= 1
                        fillq.pop(0)()
                while extras:
                    extras.pop(0)()
                for av_u in avs[1:]:
                    full_av(av_u)

            for u in range(4, 16):
                emit_unit(u)
            # drain: flush pending fills (incl. tp(0,3)), then pipeline the
            # last unit per qtile: AV group -> transpose column -> proj tile.
            while fillq:
                fillq.pop(0)()

            def tp_g(g):
                tps = psmm.tile([128, 128], F16, tag="mm", name=f"tpg{g}")
                nc.tensor.transpose(tps[:], osb_tiles[(1, 3, g)][:], iden[:])
                nc.vector.tensor_copy(
                    ot[1][:, 1536 + g * 128:1536 + (g + 1) * 128], tps[:])

            # interleave so each PE step has other work between the
            # cross-engine dependency hops (divide -> transpose -> proj)
            av_group(3, 3, 0)
            av_group(3, 3, 1)
            tp_g(0)
            av_group(3, 3, 2)
            proj_tile(12, use_act=True)
            tp_g(1)
            av_group(3, 3, 3)
            proj_tile(13, use_act=True)
            tp_g(2)
            proj_tile(14, use_act=True)
            tp_g(3)
            proj_tile(15, use_act=True)

    nc.compile()
    return nc


_CACHE = {}


def _get_nc():
    if "nc" not in _CACHE:
        _CACHE["nc"] = build_bass()
    return _CACHE["nc"]


def _pack8(w):
    """[1024, n] -> [128, 8*n] with [p, ci*n+j] = w[ci*128+p, j]"""
    n = w.shape[1]
    return np.ascontiguousarray(
        w.reshape(8, 128, n).transpose(1, 0, 2).reshape(128, 8 * n))


def make_in_maps(x, w_qkv, b_qkv, w_proj):
    iden = np.eye(128, dtype=np.float16)
    in_maps = []
    for core in range(N_CORES):
        b = core // 4
        hg = core % 4
        cs = slice(hg * DL, (hg + 1) * DL)
        wq = w_qkv[:, 0 * C:1 * C][:, cs].astype(np.float16)
        wk = w_qkv[:, 1 * C:2 * C][:, cs].astype(np.float16)
        wv = w_qkv[:, 2 * C:3 * C][:, cs].astype(np.float16)
        bq = b_qkv[0 * C:1 * C][cs].astype(np.float32)
        xT = np.ascontiguousarray(x[b].T).astype(np.float16)   # [C, T]
        wp2 = w_proj[cs, :].astype(np.float16)                 # [256, 1024]
        wp_pack = np.ascontiguousarray(
            wp2.reshape(2, 128, C).transpose(1, 0, 2).reshape(128, 2 * C))
        in_maps.append({
            "x_pack": _pack8(xT),
            "wk_pack": _pack8(wk),
            "wq_pack": _pack8(wq),
            "wv_pack": _pack8(wv),
            "wp_pack": wp_pack,
            "b_q": np.stack([bq[0:128], bq[128:256]], axis=1),
            "iden16": iden,
        })
    return in_maps


def kernel(x, w_qkv, b_qkv, w_proj, b_proj, **runner_kwargs):
    x = np.asarray(x, dtype=np.float32)
    w_qkv = np.asarray(w_qkv, dtype=np.float32)
    b_qkv = np.asarray(b_qkv, dtype=np.float32)
    w_proj = np.asarray(w_proj, dtype=np.float32)
    b_proj = np.asarray(b_proj, dtype=np.float32)

    nc = _get_nc()
    in_maps = make_in_maps(x, w_qkv, b_qkv, w_proj)
    res = run_bass_kernel_spmd(nc, in_maps, list(range(N_CORES)), **runner_kwargs)
    parts = [res.results[i]["out_partial"] for i in range(N_CORES)]
    # fold V bias through the projection; K bias is softmax-invariant
    b_eff = b_proj + b_qkv[2 * C:3 * C].astype(np.float64) @ w_proj.astype(np.float64)
    outv = np.zeros((B, T, C), dtype=np.float32)
    for b in range(B):
        for hg in range(4):
            outv[b] += parts[4 * b + hg].astype(np.float32)
        outv[b] += b_eff.astype(np.float32)[None, :]
    if runner_kwargs:
        return outv, res
    return outv


if __name__ == "__main__":
    import reference

    inputs = reference.setup_inputs()
    inputs = {k: np.asarray(v) for k, v in inputs.items()}
    got = kernel(**inputs)
    want = np.asarray(reference.reference(**inputs))
    err = np.abs(got - want).max() / np.abs(want).max()
    print("rel err:", err)


# revision 23
# speedup vs baseline: 1.0212x; 1.0004x over previous
"""Multi-head self-attention Trainium2 kernel, sharded over 8 NeuronCores.

Sharding: core = (batch, head_group): 2 batches x 4 head-groups (4 heads each).
Each core computes qkv for its batch restricted to its heads, full-sequence
attention for those heads, and a row-parallel slice of the output projection,
producing a partial [T, C] output (fp16). Host: out[b] = sum of the 4
head-group partials + b_eff where b_eff folds b_proj and the V bias.

v2 design notes (all relative to the fp32/on-chip-transpose baseline):
  - x is transposed, packed and cast to fp16 on the host; no on-chip
    transposes or x^T copies are needed.
  - K bias is dropped entirely (softmax is invariant to per-query constants,
    and q.bk is per-query); V bias is folded into b_proj on the host
    (sum_s w_s = 1); only the Q bias is applied on-chip.
  - AV is computed transposed: out[q, d] = sum_s P[s,q] V[s,d] with
    ap_size=65 per chunk matmul, which halves the PE cost of AV and makes
    the softmax divide a single per-partition tensor_scalar divide.
  - The softmax denominator comes from a ones-column appended per head in
    the V tile (memset once).
  - Everything on the PE runs fp16 (1.0 cycles/row); fp8 was measured to
    break the 2e-2 gate (diffuse attention preserves per-key noise).
"""

import math
import sys

import numpy as np

sys.path.insert(0, "/opt/trn_rl_repo")

import concourse.bacc as bacc
import concourse.bass as bass
import concourse.tile as tile
from concourse import mybir
from concourse.bass_utils import run_bass_kernel_spmd

B, T, C = 2, 2048, 1024
NH, DH = 16, 64
HG = 4                  # heads per core
DL = HG * DH            # 256 local head dims
N_CORES = 8

F32 = mybir.dt.float32
F16 = mybir.dt.float16

SCALE = 1.0 / math.sqrt(DH)
Exp = mybir.ActivationFunctionType.Exp


def build_bass():
    nc = bacc.Bacc("TRN2", target_bir_lowering=False, debug=False)

    # host-packed params: [p, ci*w + j] = w[ci*128 + p, j]
    x_in = nc.declare_dram_parameter("x_pack", [128, 8 * T], F16, isOutput=False)
    wk_in = nc.declare_dram_parameter("wk_pack", [128, 8 * DL], F16, isOutput=False)
    wq_in = nc.declare_dram_parameter("wq_pack", [128, 8 * DL], F16, isOutput=False)
    wv_in = nc.declare_dram_parameter("wv_pack", [128, 8 * DL], F16, isOutput=False)
    wp_in = nc.declare_dram_parameter("wp_pack", [128, 2 * C], F16, isOutput=False)
    bq_in = nc.declare_dram_parameter("b_q", [128, 2], F32, isOutput=False)
    id_in = nc.declare_dram_parameter("iden16", [128, 128], F16, isOutput=False)
    out = nc.declare_dram_parameter("out_partial", [T, C], F16, isOutput=True)

    with tile.TileContext(nc) as tc:
        with (
            tc.tile_pool(name="singles", bufs=1) as singles,
            tc.tile_pool(name="pt", bufs=44) as ptp,
            tc.tile_pool(name="osb", bufs=6) as osbp,
            tc.tile_pool(name="oout", bufs=3) as ooutp,
            tc.tile_pool(name="sc", bufs=2, space="PSUM") as pssc,     # 2x2 banks
            tc.tile_pool(name="avp", bufs=1, space="PSUM") as psav,    # 1 bank
            tc.tile_pool(name="mm", bufs=3, space="PSUM") as psmm,     # 3x1 bank
        ):
            # ---- persistent sbuf tiles ---------------------------------
            warm = singles.tile([128, 512], F16, name="warm")
            nc.vector.memset(warm[:], 0.0)
            # pre-load the Exp activation table while DMAs are in flight
            warm_exp = singles.tile([128, 1], F16, name="warm_exp")
            nc.scalar.activation(warm_exp[:], warm[:, 0:1], Exp, scale=SCALE)

            xt = singles.tile([128, 8 * T], F16, name="xt")
            xt3 = xt[:].rearrange("p (ci t) -> p ci t", ci=8)
            xsrc = x_in[:].rearrange("p (ci t) -> p ci t", ci=8)
            NSL = 8
            TSL = T // NSL

            def x_slice(s):
                nc.sync.dma_start(
                    out=xt3[:, :, s * TSL:(s + 1) * TSL],
                    in_=xsrc[:, :, s * TSL:(s + 1) * TSL],
                )

            # DMA order tuned so the first score tile unblocks earliest:
            # Q projection (wq + x s0,s1) is the long pole for score p0.
            wq = singles.tile([128, 8 * DL], F16, name="wq")
            nc.sync.dma_start(out=wq[:], in_=wq_in[:])
            bq = singles.tile([128, 2], F32, name="bq")
            nc.sync.dma_start(out=bq[:], in_=bq_in[:])
            x_slice(0)
            x_slice(1)
            wk = singles.tile([128, 8 * DL], F16, name="wk")
            nc.sync.dma_start(out=wk[:], in_=wk_in[:])
            x_slice(2)
            x_slice(3)
            wv = singles.tile([128, 8 * DL], F16, name="wv")
            nc.sync.dma_start(out=wv[:], in_=wv_in[:])
            x_slice(4)
            x_slice(5)
            x_slice(6)
            x_slice(7)
            wp = singles.tile([128, 2 * C], F16, name="wp")
            nc.sync.dma_start(out=wp[:], in_=wp_in[:])
            iden = singles.tile([128, 128], F16, name="iden")
            nc.sync.dma_start(out=iden[:], in_=id_in[:])

            qt = [singles.tile([128, T], F16, name=f"qt{m}") for m in range(2)]
            kt = [singles.tile([128, T], F16, name=f"kt{m}") for m in range(2)]
            v_sb = [singles.tile([128, HG * (DH + 1)], F16, name=f"v{tt}")
                    for tt in range(16)]
            for tt in range(16):
                nc.vector.memset(v_sb[tt][:, DH:HG * (DH + 1):DH + 1], 1.0)
            ot = [singles.tile([128, T], F16, name=f"ot{hp}") for hp in range(2)]

            # ---- PE warmup: chew through the pstate ramp while DMAs land
            for i in range(8):
                wps = psmm.tile([128, 512], F32, tag="mm", name=f"warm{i}")
                nc.tensor.matmul(wps[:], lhsT=warm[:, 0:128], rhs=warm[:],
                                 start=True, stop=True)

            # ---- building blocks ---------------------------------------
            def k_block(km, tb):
                """K projection for 512 tokens -> kt[km][:, tb*512:...]

                Two half tiles so the psum->sbuf copy of the first 256
                tokens overlaps the second half's matmuls."""
                for half in range(2):
                    s = 2 * tb + half
                    ps = psmm.tile([128, 256], F32, tag="mm", name=f"k{km}_{s}")
                    for ci in range(8):
                        nc.tensor.matmul(
                            ps[:],
                            lhsT=wk[:, ci * 256 + km * 128: ci * 256 + (km + 1) * 128],
                            rhs=xt3[:, ci, s * 256:(s + 1) * 256],
                            start=(ci == 0),
                            stop=(ci == 7),
                        )
                    nc.vector.tensor_copy(kt[km][:, s * 256:(s + 1) * 256], ps[:])

            def q_block(qm, tb):
                for half in range(2):
                    s = 2 * tb + half
                    ps = psmm.tile([128, 256], F32, tag="mm", name=f"q{qm}_{s}")
                    for ci in range(8):
                        nc.tensor.matmul(
                            ps[:],
                            lhsT=wq[:, ci * 256 + qm * 128: ci * 256 + (qm + 1) * 128],
                            rhs=xt3[:, ci, s * 256:(s + 1) * 256],
                            start=(ci == 0),
                            stop=(ci == 7),
                        )
                    nc.vector.tensor_scalar_add(
                        qt[qm][:, s * 256:(s + 1) * 256], ps[:], bq[:, qm:qm + 1])

            def v_block(tt):
                """V projection for 128 tokens -> v_sb[tt] (65-col head blocks)"""
                ps = psmm.tile([128, 256], F32, tag="mm", name=f"v{tt}")
                for ci in range(8):
                    nc.tensor.matmul(
                        ps[:],
                        lhsT=xt3[:, ci, tt * 128:(tt + 1) * 128],
                        rhs=wv[:, ci * 256:(ci + 1) * 256],
                        start=(ci == 0),
                        stop=(ci == 7),
                    )
                dst = v_sb[tt][:].rearrange("p (h c) -> p h c", h=HG)[:, :, 0:DH]
                src = ps[:].rearrange("p (h c) -> p h c", h=HG)
                nc.vector.tensor_copy(dst, src)

            # scores tile p of unit (h, qb): key chunks 2p,2p+1 x 512 queries
            pt_tiles = {}

            def sc_tile(h, qb, p):
                km = h // 2
                row = (h % 2) * 64
                ps = pssc.tile([128, 1024], F32, tag="sc", name=f"s{h}_{qb}_{p}")
                for half in range(2):
                    st = 2 * p + half
                    nc.tensor.matmul(
                        ps[:, half * 512:(half + 1) * 512],
                        lhsT=kt[km][row:row + 64, st * 128:(st + 1) * 128],
                        rhs=qt[km][row:row + 64, qb * 512:(qb + 1) * 512],
                        start=True,
                        stop=True,
                    )
                pt = ptp.tile([128, 1024], F16, tag="pt", name=f"p{h}_{qb}_{p}")
                nc.scalar.activation(pt[:], ps[:], Exp, scale=SCALE)
                pt_tiles[(h, qb, p)] = pt

            osb_tiles = {}
            # one PSUM bank holds 4 rotating 65-col AV slots
            av_all = psav.tile([128, 4 * (DH + 1)], F32, name="av_all")
            av_ctr = [0]

            def av_group(h, qb, g):
                """AV^T for queries qtile=qb*4+g of head h -> divide into osb."""
                hp, col = h // 2, (h % 2) * 64
                slot = av_ctr[0] % 4
                av_ctr[0] += 1
                av = av_all[:, slot * (DH + 1):(slot + 1) * (DH + 1)]
                for st in range(16):
                    ptk = pt_tiles[(h, qb, st // 2)]
                    nc.tensor.matmul(
                        av[:],
                        lhsT=ptk[:, (st % 2) * 512 + g * 128:
                                 (st % 2) * 512 + (g + 1) * 128],
                        rhs=v_sb[st][:, h * (DH + 1):(h + 1) * (DH + 1)],
                        start=(st == 0),
                        stop=(st == 15),
                    )
                key = (hp, qb, g)
                if key not in osb_tiles:
                    osb_tiles[key] = osbp.tile([128, 128], F16, tag="osb",
                                               name=f"o{hp}_{qb}_{g}")
                rec = osbp.tile([128, 1], F32, tag="rec", bufs=4,
                                name=f"r{h}_{qb}_{g}")
                nc.vector.reciprocal(rec[:], av[:, DH:DH + 1])
                nc.vector.tensor_scalar_mul(
                    osb_tiles[key][:, col:col + 64], av[:, 0:DH], rec[:, 0:1])

            def transpose_hp(hp, qb):
                """osb pair tiles (4 qtiles) -> ot[hp][:, qb*512:...]"""
                ps = psmm.tile([128, 512], F16, tag="mm", name=f"t{hp}_{qb}")
                for g in range(4):
                    nc.tensor.transpose(
                        ps[:, g * 128:(g + 1) * 128],
                        osb_tiles[(hp, qb, g)][:],
                        iden[:],
                    )
                nc.vector.tensor_copy(ot[hp][:, qb * 512:(qb + 1) * 512], ps[:])

            Copy = mybir.ActivationFunctionType.Copy

            def proj_tile(tt, use_act=False):
                o_out = ooutp.tile([128, C], F16, tag="oout", name=f"oo{tt}")
                for nb in range(2):
                    ps = psmm.tile([128, 512], F32, tag="mm", name=f"pr{tt}_{nb}")
                    for hp in range(2):
                        nc.tensor.matmul(
                            ps[:],
                            lhsT=ot[hp][:, tt * 128:(tt + 1) * 128],
                            rhs=wp[:, hp * C + nb * 512: hp * C + (nb + 1) * 512],
                            start=(hp == 0),
                            stop=(hp == 1),
                        )
                    dst = o_out[:, nb * 512:(nb + 1) * 512]
                    if use_act:
                        # tail: ACT is idle after the last exp, DVE is not;
                        # half-DMAs overlap the copy of the other half
                        nc.scalar.activation(dst, ps[:], Copy)
                        nc.sync.dma_start(
                            out=out[tt * 128:(tt + 1) * 128,
                                    nb * 512:(nb + 1) * 512],
                            in_=dst)
                    else:
                        nc.vector.tensor_copy(dst, ps[:])
                if not use_act:
                    nc.sync.dma_start(out=out[tt * 128:(tt + 1) * 128, :],
                                      in_=o_out[:])

            # ---- fill queue: transposes + proj consumed in spare PE slots
            fillq = []
            done_av = set()

            def maybe_posts(av_u):
                qb, h = av_u // 4, av_u % 4
                if h == 1:
                    fillq.append(lambda qb=qb: transpose_hp(0, qb))
                elif h == 3 and qb < 3:
                    fillq.append(lambda qb=qb: transpose_hp(1, qb))
                    for tt in range(4 * qb, 4 * qb + 4):
                        fillq.append(lambda tt=tt: proj_tile(tt))

            def full_av(av_u):
                for g in range(4):
                    av_group(av_u % 4, av_u // 4, g)
                done_av.add(av_u)
                maybe_posts(av_u)

            # ---- intro: K + qb0 scores dominate; only 12 V blocks and the
            # qb0 Q blocks live here so ACT stays fed from the start.
            # heads 0,1 share K/Q m-block 0, so their 4 score tiles can all
            # fire right after K0 (+Q0); K1/Q1/V hide under those exps.
            IV = {0: [0], 1: [1, 2, 3], 2: [4, 5, 6], 3: [7, 8, 9, 10, 11]}
            for tb in range(4):
                vq = list(IV[tb])
                if tb == 0:
                    q_block(0, 0)
                k_block(0, tb)
                sc_tile(0, 0, 2 * tb)
                sc_tile(0, 0, 2 * tb + 1)
                sc_tile(1, 0, 2 * tb)
                sc_tile(1, 0, 2 * tb + 1)
                k_block(1, tb)
                if tb == 0:
                    q_block(1, 0)
                if vq:
                    v_block(vq.pop(0))
                sc_tile(2, 0, 2 * tb)
                if vq:
                    v_block(vq.pop(0))
                sc_tile(2, 0, 2 * tb + 1)
                if tb == 3:
                    q_block(0, 1)
                sc_tile(3, 0, 2 * tb)
                if vq:
                    v_block(vq.pop(0))
                sc_tile(3, 0, 2 * tb + 1)
                while vq:
                    v_block(vq.pop(0))

            # ---- steady state: units u = qb*4 + h -----------------------
            pre_fills = {}
            mid_fills = {5: [lambda: q_block(1, 1)],
                         6: [lambda: q_block(0, 2)],
                         7: [lambda: q_block(1, 2)],
                         8: [lambda: q_block(0, 3)],
                         9: [lambda: q_block(1, 3)]}
            unit_v = {4: [12, 13, 14, 15]}
            av_plan = {5: [0], 6: [1], 7: [2, 3], 8: [4, 5], 9: [6, 7],
                       10: [8], 11: [9], 12: [10], 13: [11], 14: [12],
                       15: [13, 14]}

            def emit_unit(u):
                qb, h = u // 4, u % 4
                for f in pre_fills.get(u, []):
                    f()
                avs = av_plan.get(u, [])
                first = avs[0] if avs else None
                extras = [lambda tt=tt: v_block(tt) for tt in unit_v.get(u, [])]
                extras += mid_fills.get(u, [])
                nfill = 2
                for p in range(8):
                    sc_tile(h, qb, p)
                    if first is not None and 2 <= p <= 5:
                        g = p - 2
                        av_group(first % 4, first // 4, g)
                        if g == 3:
                            done_av.add(first)
                            maybe_posts(first)
                    elif extras:
                        extras.pop(0)()
                    elif nfill and fillq:
                        nfill -= 1
                        fillq.pop(0)()
                while extras:
                    extras.pop(0)()
                for av_u in avs[1:]:
                    full_av(av_u)

            for u in range(4, 16):
                emit_unit(u)
            # drain: flush pending fills (incl. tp(0,3)), then pipeline the
            # last unit per qtile: AV group -> transpose column -> proj tile.
            while fillq:
                fillq.pop(0)()

            def tp_g(g):
                tps = psmm.tile([128, 128], F16, tag="mm", name=f"tpg{g}")
                nc.tensor.transpose(tps[:], osb_tiles[(1, 3, g)][:], iden[:])
                nc.vector.tensor_copy(
                    ot[1][:, 1536 + g * 128:1536 + (g + 1) * 128], tps[:])

            # interleave so each PE step has other work between the
            # cross-engine dependency hops (divide -> transpose -> proj)
            av_group(3, 3, 0)
            av_group(3, 3, 1)
            tp_g(0)
            av_group(3, 3, 2)
            proj_tile(12, use_act=True)
            tp_g(1)
            av_group(3, 3, 3)
            proj_tile(13, use_act=True)
            tp_g(2)
            proj_tile(14, use_act=True)
            tp_g(3)
            proj_tile(15, use_act=True)

    nc.compile()
    return nc


_CACHE = {}


def _get_nc():
    if "nc" not in _CACHE:
        _CACHE["nc"] = build_bass()
    return _CACHE["nc"]


def _pack8(w):
    """[1024, n] -> [128, 8*n] with [p, ci*n+j] = w[ci*128+p, j]"""
    n = w.shape[1]
    return np.ascontiguousarray(
        w.reshape(8, 128, n).transpose(1, 0, 2).reshape(128, 8 * n))


def make_in_maps(x, w_qkv, b_qkv, w_proj):
    iden = np.eye(128, dtype=np.float16)
    in_maps = []
    for core in range(N_CORES):
        b = core // 4
        hg = core % 4
        cs = slice(hg * DL, (hg + 1) * DL)
        wq = w_qkv[:, 0 * C:1 * C][:, cs].astype(np.float16)
        wk = w_qkv[:, 1 * C:2 * C][:, cs].astype(np.float16)
        wv = w_qkv[:, 2 * C:3 * C][:, cs].astype(np.float16)
        bq = b_qkv[0 * C:1 * C][cs].astype(np.float32)
        xT = np.ascontiguousarray(x[b].T).astype(np.float16)   # [C, T]
        wp2 = w_proj[cs, :].astype(np.float16)                 # [256, 1024]
        wp_pack = np.ascontiguousarray(
            wp2.reshape(2, 128, C).transpose(1, 0, 2).reshape(128, 2 * C))
        in_maps.append({
            "x_pack": _pack8(xT),
            "wk_pack": _pack8(wk),
            "wq_pack": _pack8(wq),
            "wv_pack": _pack8(wv),
            "wp_pack": wp_pack,
            "b_q": np.stack([bq[0:128], bq[128:256]], axis=1),
            "iden16": iden,
        })
    return in_maps


def kernel(x, w_qkv, b_qkv, w_proj, b_proj, **runner_kwargs):
    x = np.asarray(x, dtype=np.float32)
    w_qkv = np.asarray(w_qkv, dtype=np.float32)
    b_qkv = np.asarray(b_qkv, dtype=np.float32)
    w_proj = np.asarray(w_proj, dtype=np.float32)
    b_proj = np.asarray(b_proj, dtype=np.float32)

    nc = _get_nc()
    in_maps = make_in_maps(x, w_qkv, b_qkv, w_proj)
    res = run_bass_kernel_spmd(nc, in_maps, list(range(N_CORES)), **runner_kwargs)
    parts = [res.results[i]["out_partial"] for i in range(N_CORES)]
    # fold V bias through the projection; K bias is softmax-invariant
    b_eff = b_proj + b_qkv[2 * C:3 * C].astype(np.float64) @ w_proj.astype(np.float64)
    outv = np.zeros((B, T, C), dtype=np.float32)
    for b in range(B):
        for hg in range(4):
            outv[b] += parts[4 * b + hg].astype(np.float32)
        outv[b] += b_eff.astype(np.float32)[None, :]
    if runner_kwargs:
        return outv, res
    return outv


if __name__ == "__main__":
    import reference

    inputs = reference.setup_inputs()
    inputs = {k: np.asarray(v) for k, v in inputs.items()}
    got = kernel(**inputs)
    want = np.asarray(reference.reference(**inputs))
    err = np.abs(got - want).max() / np.abs(want).max()
    print("rel err:", err)
